# revision 24
# baseline (speedup 1.0000x reference)
"""Trainium2 Bass kernel for nn_DA_conv (degradation-aware dynamic-filter conv).

kernel(**inputs) takes FULL inputs (as from setup_inputs()), shards batch
B=16 across 8 NeuronCores (2 batches/core), runs one SPMD Bass program on
cores 0-7, gathers the full [16,64,128,128] fp32 output.

v2 design (vs v1 baseline 637us):
  - host-casts x0/x2 to bf16; pad build is a pure strided DMA (no engine pass)
  - channel max/sum maps via GPSIMD partition_all_reduce (mlp library),
    replacing the DVE fold + scalar-queue staging DMAs
  - sa taps materialized as 36 rows -> ONE accumul.-free matmul per quarter
  - attp (channel-attention + residual) folded into the PSUM accumulation as
    a diagonal matmul; x0*sa folded as an identity matmul on z_sa
  - final combine is a single ACT pass (fps + bconv), output stored bf16
  - partition-swapped kcsel/ksp (b0 at rows 64+, b1 at rows 0+) so kbc and
    conv matmuls occupy disjoint PE subarrays -> 4-way concurrency
  - software pipeline: folds(ch) / taps+sa(ch-1) / ddf(ch-2) so no engine
    queue head-blocks on a cross-stage dependency
"""

import sys

sys.path.insert(0, "/opt/trn_rl_repo")

import numpy as np
import ml_dtypes

import concourse.bass as bass
import concourse.tile as tile
from concourse import bacc, bass_isa, mybir
from concourse.bass_utils import run_bass_kernel_spmd

F32 = mybir.dt.float32
BF16 = mybir.dt.bfloat16
AF = mybir.ActivationFunctionType
OP = mybir.AluOpType
RED = bass_isa.ReduceOp

B, C, H, W = 16, 64, 128, 128
KK = 9
HW = H * W
NCORES = 8
BPC = B // NCORES          # batches per core
RC = 16                    # image rows per chunk
NCH = H // RC              # 8 chunks
F = RC * W                 # 2048 pixels per chunk
HF = F // 2                # 1024 (half chunk)
Q = 512                    # psum-bank quantum
PW = 132                   # padded row width
PR = 130                   # padded rows
PADN = PR * PW
NT = 17 * PW               # flat tap window length
MS = PADN + 2 * PW         # mscr/sscr cols (flat window overread margin)

# (h2*10 + tap) indices whose z-mul reads PSUM directly (no evac)
DIRECT = {2, 7, 12, 17}


def _leaky(v):
    return np.where(v >= 0, v, 0.1 * v)


def _build_program():
    nc = bacc.Bacc("TRN2", target_bir_lowering=False, debug=False,
                   num_devices=NCORES)

    def din(name, shape, dt=F32):
        return nc.dram_tensor(name, shape, dt, kind="ExternalInput").ap()

    x0_d = din("x0b", [128, HW], BF16)
    x2_d = din("x2b", [128, HW], BF16)
    kcsel_d = din("kcsel", [128, KK * 64], BF16)
    wks1_d = din("wks1", [128, 64], BF16)
    wks2_d = din("wks2", [128, KK], BF16)
    wconv_d = din("wconv", [128, 64], BF16)
    idm_d = din("idm", [128, 64], BF16)
    attpd_d = din("attpd", [128, 64], BF16)
    selsa_d = din("selsa", [128, 64], BF16)
    wsa_d = din("wsa36", [36, 2], BF16)
    bks1_d = din("bks1", [128, 1])
    bks2_d = din("bks2", [128, 1])
    bsa_d = din("bsa", [128, 1])
    bconv_d = din("bconv", [128, 1])
    out_d = nc.dram_tensor("outb", [128, HW], BF16, kind="ExternalOutput").ap()

    with tile.TileContext(nc) as tc:
        with (
            tc.tile_pool(name="persist", bufs=1) as pp,
            tc.tile_pool(name="ring2", bufs=2) as r2,
            tc.tile_pool(name="ring3", bufs=3) as r3,
            tc.tile_pool(name="psA", bufs=3, space=bass.MemorySpace.PSUM) as psA,
            tc.tile_pool(name="psF", bufs=2, space=bass.MemorySpace.PSUM) as psF,
        ):
            pad1 = pp.tile([128, PADN], BF16)
            pad2 = pp.tile([128, PADN], BF16)
            mscr = pp.tile([128, MS], BF16)    # rows 0/64: channel max b0/b1
            sscr = pp.tile([128, MS], BF16)    # rows 0/64: channel sum b0/b1
            kcsel = pp.tile([128, KK * 64], BF16)
            wks1 = pp.tile([128, 64], BF16)
            wks2 = pp.tile([128, KK], BF16)
            wconv = pp.tile([128, 64], BF16)
            idm = pp.tile([128, 64], BF16)
            attpd = pp.tile([128, 64], BF16)
            selsa = pp.tile([128, 64], BF16)
            wsa36 = pp.tile([36, 2], BF16)
            bks1 = pp.tile([128, 1], F32)
            bks2 = pp.tile([128, 1], F32)
            bsa = pp.tile([128, 1], F32)
            bconv = pp.tile([128, 1], F32)

            sdma = nc.sync.dma_start
            pdma = nc.gpsimd.dma_start
            sdma(kcsel[:], kcsel_d[:])
            sdma(wks1[:], wks1_d[:])
            sdma(wks2[:], wks2_d[:])
            sdma(wconv[:], wconv_d[:])
            sdma(idm[:], idm_d[:])
            sdma(attpd[:], attpd_d[:])
            sdma(selsa[:], selsa_d[:])
            sdma(wsa36[:], wsa_d[:])
            sdma(bks1[:], bks1_d[:])
            sdma(bks2[:], bks2_d[:])
            sdma(bsa[:], bsa_d[:])
            sdma(bconv[:], bconv_d[:])

            p1v = pad1.rearrange("p (r w) -> p r w", w=PW)
            p2v = pad2.rearrange("p (r w) -> p r w", w=PW)

            # zero only the borders (pad interior is fully DMA-overwritten)
            nc.vector.memset(p1v[:, :, 0:2], 0.0)
            nc.vector.memset(p1v[:, :, 130:132], 0.0)
            nc.vector.memset(p1v[:, 0:1, :], 0.0)
            nc.vector.memset(p1v[:, 129:130, :], 0.0)
            nc.vector.memset(p2v[:, :, 0:3], 0.0)
            nc.vector.memset(p2v[:, :, 131:132], 0.0)
            nc.vector.memset(p2v[:, 0:1, :], 0.0)
            nc.vector.memset(p2v[:, 129:130, :], 0.0)
            # folds cover flat [PW, PW+8*RC*PW); zero row 0 + tail margin
            nc.vector.memset(mscr[:, 0:PW], 0.0)
            nc.vector.memset(mscr[:, PADN - PW:MS], 0.0)
            nc.vector.memset(sscr[:, 0:PW], 0.0)
            nc.vector.memset(sscr[:, PADN - PW:MS], 0.0)

            ksps = {}    # ksp+sa rows per chunk (ring via r3)
            taps = {}
            MM = nc.tensor.matmul

            for ch in range(NCH + 2):
                # ---- stage 1: loads + folds for chunk ch ----
                if ch < NCH:
                    r0 = RC * ch
                    csl = slice(ch * F, (ch + 1) * F)
                    base = (r0 + 1) * PW
                    n = RC * PW
                    sdma(p1v[:, r0 + 1:r0 + 17, 2:130], x0_d[:, csl])
                    sdma(p2v[:, r0 + 1:r0 + 17, 3:131], x0_d[:, csl])
                    x2t = r2.tile([128, F], BF16, tag="x2t")
                    sdma(x2t[:], x2_d[:, csl])
                    # partition_all_reduce ucode only honors base partition 0:
                    # b0 reduces in place; b1 stages down to base 0 first and
                    # the single result row is DMA'd back up to row 64.
                    nc.gpsimd.partition_all_reduce(
                        mscr[0:64, base:base + n], pad1[0:64, base:base + n],
                        64, RED.max)
                    nc.gpsimd.partition_all_reduce(
                        sscr[0:64, base:base + n], pad1[0:64, base:base + n],
                        64, RED.add)
                    stg = r2.tile([64, n], BF16, tag="stg")
                    sdma(stg[:], pad1[64:128, base:base + n])
                    mr1 = r2.tile([64, n], BF16, tag="mr1", bufs=1)
                    sr1 = r2.tile([64, n], BF16, tag="sr1", bufs=1)
                    nc.gpsimd.partition_all_reduce(mr1[:], stg[:], 64, RED.max)
                    nc.gpsimd.partition_all_reduce(sr1[:], stg[:], 64, RED.add)
                    pdma(mscr[64:65, base:base + n], mr1[0:1, :])
                    pdma(sscr[64:65, base:base + n], sr1[0:1, :])

                # ---- stage 3: tap rows for chunk ch-1 ----
                if 1 <= ch <= NCH:
                    cht = ch - 1
                    rp = RC * cht
                    tp = r2.tile([36, NT], BF16, tag="tap")
                    taps[cht] = tp
                    for j in range(3):
                        for g, src in enumerate((mscr, sscr, mscr, sscr)):
                            srow = 0 if g < 2 else 64
                            for i in range(3):
                                o = (rp + i) * PW + j + 1
                                pdma(tp[j * 12 + g * 3 + i:
                                        j * 12 + g * 3 + i + 1, :],
                                     src[srow:srow + 1, o:o + NT])

                # ---- stage 4: hidden + ksp for chunk ch ----
                if ch < NCH:
                    ksp = r3.tile([128, F], BF16, tag="ksp")
                    ksps[ch] = ksp
                    for h2 in range(2):
                        hsl = slice(h2 * HF, (h2 + 1) * HF)
                        hps = psA.tile([128, HF], F32, tag="psA")
                        for q in range(2):
                            qs = slice(q * Q, (q + 1) * Q)
                            xs = slice(h2 * HF + q * Q, h2 * HF + (q + 1) * Q)
                            MM(hps[0:64, qs], wks1[0:64, :], x2t[0:64, xs],
                               start=True, stop=True, tile_position=(0, 0))
                            MM(hps[64:128, qs], wks1[64:128, :], x2t[64:128, xs],
                               start=True, stop=True, tile_position=(64, 64))
                        hsb = r2.tile([128, HF], BF16, tag="hsb")
                        nc.scalar.activation(hsb[:], hps[:], AF.Lrelu,
                                             bias=bks1[:, 0:1], alpha=0.1)
                        kps = psA.tile([128, HF], F32, tag="psA")
                        for q in range(2):
                            qs = slice(q * Q, (q + 1) * Q)
                            MM(kps[64:73, qs], wks2[0:64, :], hsb[0:64, qs],
                               start=True, stop=True, tile_position=(0, 64))
                            MM(kps[0:9, qs], wks2[64:128, :], hsb[64:128, qs],
                               start=True, stop=True, tile_position=(64, 0))
                        nc.scalar.activation(ksp[64:73, hsl], kps[64:73, :],
                                             AF.Identity, bias=bks2[64:73, 0:1])
                        nc.scalar.activation(ksp[0:9, hsl], kps[0:9, :],
                                             AF.Identity, bias=bks2[0:9, 0:1])

                # ---- stage 5: ddf for chunk ch-2 ----
                if ch >= 2:
                    chd = ch - 2
                    rp = RC * chd
                    ksp = ksps.pop(chd)
                    outst = r2.tile([128, F], BF16, tag="outst")

                    for h2 in range(2):
                        rb = rp + 8 * h2

                        def kbc(t):
                            kps_t = psA.tile([128, HF], F32, tag="psA",
                                             name="kps_t")
                            for q in range(2):
                                qs = slice(q * Q, (q + 1) * Q)
                                xs = slice(h2 * HF + q * Q,
                                           h2 * HF + (q + 1) * Q)
                                if t < KK:
                                    ksl = slice(t * 64, (t + 1) * 64)
                                    MM(kps_t[0:64, qs], kcsel[64:73, ksl],
                                       ksp[64:73, xs], start=True, stop=True,
                                       tile_position=(64, 0))
                                    MM(kps_t[64:128, qs], kcsel[0:9, ksl],
                                       ksp[0:9, xs], start=True, stop=True,
                                       tile_position=(0, 64))
                                else:      # sa broadcast: 0.5*tanh (the
                                    # missing +0.5*x0 is folded into attpd);
                                    # tanh rows live at ksp rows 96-97 / 32-33
                                    MM(kps_t[0:64, qs], selsa[96:98, :],
                                       ksp[96:98, xs], start=True, stop=True,
                                       tile_position=(96, 0))
                                    MM(kps_t[64:128, qs], selsa[32:34, :],
                                       ksp[32:34, xs], start=True, stop=True,
                                       tile_position=(32, 64))
                            return kps_t

                        def padview(t):
                            if t < KK:
                                i, j = divmod(t, 3)
                            else:
                                i, j = 1, 1
                            if j == 1:
                                srcv, joff = p1v, 2
                            else:
                                srcv, joff = p2v, j + 2
                            return srcv[:, rb + i:rb + i + 8, joff:joff + W]

                        fq = [psF.tile([128, Q], F32, tag="fps", name="fq")
                              for _ in range(2)]
                        kb = {0: kbc(0), 1: kbc(1)}
                        for t in range(KK + 1):
                            if t + 2 <= KK:
                                kb[t + 2] = kbc(t + 2)
                            kps_t = kb.pop(t)
                            xv = padview(t)
                            z = r3.tile([128, HF], BF16, tag="z")
                            zv = z.rearrange("p (r w) -> p r w", w=W)
                            if h2 * 10 + t in DIRECT:
                                nc.vector.tensor_mul(
                                    zv, xv,
                                    kps_t.rearrange("p (r w) -> p r w", w=W))
                            else:
                                kb2 = r3.tile([128, HF], BF16, tag="kb2")
                                nc.scalar.activation(kb2[:], kps_t[:], AF.Copy)
                                nc.vector.tensor_mul(
                                    zv, xv,
                                    kb2.rearrange("p (r w) -> p r w", w=W))
                            wa = wconv if t < KK else idm
                            for q in range(2):
                                qs = slice(q * Q, (q + 1) * Q)
                                MM(fq[q][0:64, :], wa[0:64, :], z[0:64, qs],
                                   start=(t == 0), stop=False,
                                   tile_position=(0, 0), skip_group_check=True)
                                MM(fq[q][64:128, :], wa[64:128, :],
                                   z[64:128, qs], start=(t == 0), stop=False,
                                   tile_position=(64, 64),
                                   skip_group_check=True)
                        # attp residual tap (center view, diag weights)
                        for q in range(2):
                            xc = p1v[:, rb + 4 * q + 1:rb + 4 * q + 5, 2:130]
                            MM(fq[q][0:64, :], attpd[0:64, :], xc[0:64],
                               start=False, stop=True, tile_position=(0, 0),
                               skip_group_check=True)
                            MM(fq[q][64:128, :], attpd[64:128, :], xc[64:128],
                               start=False, stop=True, tile_position=(64, 64),
                               skip_group_check=True)
                        for q in range(2):
                            osl = slice(h2 * HF + q * Q, h2 * HF + (q + 1) * Q)
                            nc.scalar.activation(outst[:, osl], fq[q][:],
                                                 AF.Identity, bias=bconv[:, 0:1])
                    sdma(out_d[:, chd * F:(chd + 1) * F], outst[:])

                # ---- stage 6: sa conv for chunk ch-1 (PE+ACT emitted last so
                # the ACT queue drains the ddf evacs first) ----
                if 1 <= ch <= NCH:
                    cht = ch - 1
                    tp = taps[cht]
                    tapv = tp.rearrange("p (r w) -> p r w", w=PW)
                    kspt = ksps[cht]
                    for h2 in range(2):
                        saps = psA.tile([128, HF], F32, tag="psA")
                        for q in range(2):
                            qg = 2 * h2 + q
                            qs = slice(q * Q, (q + 1) * Q)
                            # two copies of the [2,512] sa output: partitions
                            # 96-97 (b0 lane) and 32-33 (b1 lane)
                            MM(saps[96:98, qs], wsa36[:],
                               tapv[:, 4 * qg:4 * qg + 4, 0:W],
                               start=True, stop=True, tile_position=(0, 96))
                            MM(saps[32:34, qs], wsa36[:],
                               tapv[:, 4 * qg:4 * qg + 4, 0:W],
                               start=True, stop=True, tile_position=(0, 32))
                        # sigmoid(x+b) = 0.5*tanh(0.5x+0.5b) + 0.5; the 0.5
                        # scale rides on selsa, the +0.5 on attpd
                        hsl = slice(h2 * HF, (h2 + 1) * HF)
                        nc.scalar.activation(
                            kspt[96:98, hsl], saps[96:98, :],
                            AF.Tanh, bias=bsa[96:98, 0:1], scale=0.5)
                        nc.scalar.activation(
                            kspt[32:34, hsl], saps[32:34, :],
                            AF.Tanh, bias=bsa[32:34, 0:1], scale=0.5)

    nc.compile()
    return nc


_CACHED = {}


def _get_program():
    if "nc" not in _CACHED:
        _CACHED["nc"] = _build_program()
    return _CACHED["nc"]


def make_in_maps(x0, x1, x2, Wkc1, Wkc2, Wks1, bks1, Wks2, bks2,
                 Wconv, bconv, Wca1, Wca2, Wsa, bsa):
    bf = ml_dtypes.bfloat16
    x0 = np.asarray(x0, np.float32)
    x1 = np.asarray(x1, np.float32)
    x2 = np.asarray(x2, np.float32)

    kc = (_leaky(x1 @ np.asarray(Wkc1, np.float32))
          @ np.asarray(Wkc2, np.float32)).reshape(B, C, KK)
    att = 1.0 / (1.0 + np.exp(-(_leaky(x1 @ np.asarray(Wca1, np.float32))
                                @ np.asarray(Wca2, np.float32))))
    attp = (1.0 + att).astype(np.float32)

    wks1_np = np.tile(np.asarray(Wks1, np.float32), (2, 1)).astype(bf)
    wks2_np = np.tile(np.asarray(Wks2, np.float32), (2, 1)).astype(bf)
    wconv_np = np.tile(np.asarray(Wconv, np.float32), (2, 1)).astype(bf)
    idm_np = np.tile(np.eye(64, dtype=np.float32), (2, 1)).astype(bf)
    # sa = 0.5*tanh + 0.5; the broadcast applies 0.5*tanh, the +0.5*x0
    # remainder rides on the attp diagonal below
    selsa_np = np.zeros((128, 64), np.float32)
    selsa_np[96, :] = 0.5        # b0 lane: ksp row 96 (tanh b0)
    selsa_np[33, :] = 0.5        # b1 lane: ksp row 33 (tanh b1)
    selsa_np = selsa_np.astype(bf)

    wsa_np = np.asarray(Wsa, np.float32)[0]          # [2 src, 3 i, 3 j]
    w36 = np.zeros((36, 2), np.float32)
    for j in range(3):
        for g in range(4):
            b, srcm = divmod(g, 2)
            s = (1.0 / 64.0) if srcm == 1 else 1.0
            for i in range(3):
                w36[j * 12 + g * 3 + i, b] = wsa_np[srcm, i, j] * s
    w36 = w36.astype(bf)

    bks1_np = np.tile(np.asarray(bks1, np.float32), 2).reshape(128, 1)
    bks2_np = np.zeros((128, 1), np.float32)
    bks2_np[0:9, 0] = np.asarray(bks2, np.float32)
    bks2_np[64:73, 0] = np.asarray(bks2, np.float32)
    bsa_np = np.full((128, 1), 0.5 * float(np.asarray(bsa, np.float32)[0]),
                     np.float32)
    bconv_np = np.ascontiguousarray(
        np.tile(np.asarray(bconv, np.float32), 2).reshape(128, 1))

    shared = {
        "wks1": wks1_np, "wks2": wks2_np, "wconv": wconv_np, "idm": idm_np,
        "selsa": selsa_np, "wsa36": w36, "bks1": bks1_np, "bks2": bks2_np,
        "bsa": bsa_np, "bconv": bconv_np,
    }

    in_maps = []
    for cid in range(NCORES):
        bsl = slice(BPC * cid, BPC * (cid + 1))
        m = dict(shared)
        m["x0b"] = np.ascontiguousarray(
            x0[bsl].reshape(128, HW).astype(bf))
        m["x2b"] = np.ascontiguousarray(
            x2[bsl].reshape(128, HW).astype(bf))
        # kc selector: b0 one-hot rows at partitions 64-72, b1 at 0-8
        kcs = np.zeros((128, KK * 64), np.float32)
        for ij in range(KK):
            kcs[64 + ij, ij * 64:(ij + 1) * 64] = kc[BPC * cid, :, ij]
            kcs[ij, ij * 64:(ij + 1) * 64] = kc[BPC * cid + 1, :, ij]
        m["kcsel"] = kcs.astype(bf)
        ad = np.zeros((128, 64), np.float32)
        ad[np.arange(64), np.arange(64)] = attp[BPC * cid] + 0.5
        ad[64 + np.arange(64), np.arange(64)] = attp[BPC * cid + 1] + 0.5
        m["attpd"] = ad.astype(bf)
        in_maps.append(m)
    return in_maps


def kernel(**inputs):
    in_maps = make_in_maps(**inputs)
    nc = _get_program()
    res = run_bass_kernel_spmd(nc, in_maps, list(range(NCORES)))
    out = np.empty((B, C, H, W), np.float32)
    for cid in range(NCORES):
        out[BPC * cid:BPC * (cid + 1)] = \
            res.results[cid]["outb"].astype(np.float32).reshape(BPC, C, H, W)
    return out


if __name__ == "__main__":
    _get_program()
    print("program built and compiled OK")


# revision 28
# speedup vs baseline: 1.5498x; 1.5498x over previous
"""Trainium2 Bass kernel for nn_DA_conv (degradation-aware dynamic-filter conv).

kernel(**inputs) takes FULL inputs (as from setup_inputs()), shards batch
B=16 across 8 NeuronCores (2 batches/core), runs one SPMD Bass program on
cores 0-7, gathers the full [16,64,128,128] fp32 output.

v2.2 design (vs v1 baseline 637us, v2.0 765us):
  - host-casts x0/x2 to bf16; pad build is a pure strided DMA; out is bf16
  - channel max via a DVE fold over 2-chunk column spans (6 staging DMAs
    per chunk); channel sum via one PE ones-matmul pair + DVE evac
  - single combined max/sum map tile (rows 0/1 av, 2/3 mx)
  - sa conv as one 36-row matmul per quarter; taps built by 4 DMAs/chunk
    with hand-built overlapping strided APs; sigmoid via tanh (keeps all
    activations in one ACT table set)
  - ksp bias folded into kcsel via a ones-row (contraction 10)
  - x0*(1.5+att+0.5tanh) fused as one STT (t2); final = STT(fps+bconv)+t2
  - partition-swapped kcsel/ksp so kbc and conv matmuls occupy disjoint
    PE subarrays; software pipeline ddf(ch-3)/taps+sa(ch-2)/fold+loads(ch)
"""

import sys

sys.path.insert(0, "/opt/trn_rl_repo")

import dataclasses

import numpy as np
import ml_dtypes

import concourse.bass as bass
import concourse.tile as tile
from concourse import bacc, mybir
from concourse.bass_utils import run_bass_kernel_spmd

F32 = mybir.dt.float32
BF16 = mybir.dt.bfloat16
AF = mybir.ActivationFunctionType
OP = mybir.AluOpType

B, C, H, W = 16, 64, 128, 128
KK = 9
HW = H * W
NCORES = 8
BPC = B // NCORES          # batches per core
RC = 16                    # image rows per chunk
NCH = H // RC              # 8 chunks
F = RC * W                 # 2048 pixels per chunk
HF = F // 2                # 1024 (half chunk)
Q = 512                    # psum-bank quantum
PW = 132                   # padded row width
PR = 130                   # padded rows
PADN = PR * PW
N2 = 2 * RC * PW           # fold-pair column span (4224)
NT = 18 * PW               # flat tap window length (row-divisible)
MS = PADN + 2 * PW         # map tile cols (flat window overread margin)

# instance engine assignment, idx = h2*9 + t (t in 0..8):
GP_MUL = {0, 3, 6, 10, 13, 16}     # ACT-evac + GPSIMD mul
ACT_DVE = {1, 7, 12}               # ACT-evac + DVE mul
# rest: DVE direct mul from PSUM


def _leaky(v):
    return np.where(v >= 0, v, 0.1 * v)


def _build_program():
    nc = bacc.Bacc("TRN2", target_bir_lowering=False, debug=False,
                   num_devices=NCORES)

    def din(name, shape, dt=F32):
        return nc.dram_tensor(name, shape, dt, kind="ExternalInput").ap()

    x0_d = din("x0b", [128, HW], BF16)
    x2_d = din("x2b", [128, HW], BF16)
    kcsel_d = din("kcsel", [128, KK * 64], BF16)
    wks1_d = din("wks1", [128, 64], BF16)
    wks2_d = din("wks2", [128, KK], BF16)
    wconv_d = din("wconv", [128, 64], BF16)
    selsa_d = din("selsa", [128, 128], BF16)
    wmean_d = din("wmean2", [128, 2], BF16)
    wsa_d = din("wsa12", [12, 6], BF16)
    bks1_d = din("bks1", [128, 1])
    bks2_d = din("bks2", [128, 1])
    bsa_d = din("bsa", [128, 1])
    bconv_d = din("bconv", [128, 1])
    attps_d = din("attps", [128, 1])
    out_d = nc.dram_tensor("outb", [128, HW], BF16, kind="ExternalOutput").ap()

    with tile.TileContext(nc) as tc:
        with (
            tc.tile_pool(name="persist", bufs=1) as pp,
            tc.tile_pool(name="ring2", bufs=2) as r2,
            tc.tile_pool(name="ring3", bufs=3) as r3,
            tc.tile_pool(name="ring4", bufs=4) as r4,
            tc.tile_pool(name="psA", bufs=3, space=bass.MemorySpace.PSUM) as psA,
            tc.tile_pool(name="psF", bufs=2, space=bass.MemorySpace.PSUM) as psF,
        ):
            pad1 = pp.tile([128, PADN], BF16)
            pad2 = pp.tile([128, PADN], BF16)
            # combined maps: rows 0/1 = channel-sum b0/b1, 2/3 = max b0/b1
            mssc = pp.tile([128, MS], BF16)
            vs = pp.tile([128, N2], BF16)      # fold staging
            sA = pp.tile([128, N2], BF16)      # fold ping
            sB = pp.tile([128, N2], BF16)      # fold pong
            kcsel = pp.tile([128, KK * 64], BF16)
            wks1 = pp.tile([128, 64], BF16)
            wks2 = pp.tile([128, KK], BF16)
            wconv = pp.tile([128, 64], BF16)
            selsa = pp.tile([128, 128], BF16)
            wmean2 = pp.tile([128, 2], BF16)
            wsa12 = pp.tile([12, 6], BF16)
            bks1 = pp.tile([128, 1], F32)
            bks2 = pp.tile([128, 1], F32)
            bsa = pp.tile([128, 1], F32)
            bconv = pp.tile([128, 1], F32)
            attps = pp.tile([128, 1], F32)

            sdma = nc.sync.dma_start
            pdma = nc.gpsimd.dma_start
            sdma(kcsel[:], kcsel_d[:])
            sdma(wks1[:], wks1_d[:])
            sdma(wks2[:], wks2_d[:])
            sdma(wconv[:], wconv_d[:])
            sdma(selsa[:], selsa_d[:])
            sdma(wmean2[:], wmean_d[:])
            sdma(wsa12[:], wsa_d[:])
            sdma(bks1[:], bks1_d[:])
            sdma(bks2[:], bks2_d[:])
            sdma(bsa[:], bsa_d[:])
            sdma(bconv[:], bconv_d[:])
            sdma(attps[:], attps_d[:])

            p1v = pad1.rearrange("p (r w) -> p r w", w=PW)
            p2v = pad2.rearrange("p (r w) -> p r w", w=PW)
            mscv = mssc.rearrange("p (r w) -> p r w", w=PW)

            # zero pad borders (interior is fully DMA-overwritten)
            nc.vector.memset(p1v[:, :, 0:2], 0.0)
            nc.vector.memset(p1v[:, :, 130:132], 0.0)
            nc.vector.memset(p1v[:, 0:1, :], 0.0)
            nc.vector.memset(p1v[:, 129:130, :], 0.0)
            nc.vector.memset(p2v[:, :, 0:3], 0.0)
            nc.vector.memset(p2v[:, :, 131:132], 0.0)
            nc.vector.memset(p2v[:, 0:1, :], 0.0)
            nc.vector.memset(p2v[:, 129:130, :], 0.0)
            nc.vector.memset(mssc[0:4, :], 0.0)
            nc.vector.memset(vs[:], 0.0)

            ksps = {}
            taps = {}
            MM = nc.tensor.matmul

            def strided_src(t, row, off, dims):
                a = t[row:row + 1, off:off + 1]
                return dataclasses.replace(a, ap=[[t.ap[0][0], 1]] + dims)

            for ch in range(NCH + 3):
                # ---- loads + hidden/ksp + mean for chunk ch ----
                if ch < NCH:
                    r0 = RC * ch
                    csl = slice(ch * F, (ch + 1) * F)
                    sdma(p1v[:, r0 + 1:r0 + 17, 2:130], x0_d[:, csl])
                    sdma(p2v[:, r0 + 1:r0 + 17, 3:131], x0_d[:, csl])
                    x2t = r2.tile([128, F], BF16, tag="x2t")
                    sdma(x2t[:], x2_d[:, csl])

                    ksp = r4.tile([128, F], BF16, tag="ksp")
                    ksps[ch] = ksp
                    for h2 in range(2):
                        hsl = slice(h2 * HF, (h2 + 1) * HF)
                        hps = psA.tile([128, HF], F32, tag="psA", name="hps")
                        for q in range(2):
                            qs = slice(q * Q, (q + 1) * Q)
                            xs = slice(h2 * HF + q * Q, h2 * HF + (q + 1) * Q)
                            MM(hps[0:64, qs], wks1[0:64, :], x2t[0:64, xs],
                               start=True, stop=True, tile_position=(0, 0))
                            MM(hps[64:128, qs], wks1[64:128, :],
                               x2t[64:128, xs], start=True, stop=True,
                               tile_position=(64, 64))
                        hsb = r2.tile([128, HF], BF16, tag="hsb")
                        nc.scalar.activation(hsb[:], hps[:], AF.Lrelu,
                                             bias=bks1[:, 0:1], alpha=0.1)
                        kps = psA.tile([128, HF], F32, tag="psA", name="kps")
                        for q in range(2):
                            qs = slice(q * Q, (q + 1) * Q)
                            MM(kps[64:73, qs], wks2[0:64, :], hsb[0:64, qs],
                               start=True, stop=True, tile_position=(0, 64))
                            MM(kps[0:9, qs], wks2[64:128, :], hsb[64:128, qs],
                               start=True, stop=True, tile_position=(64, 0))
                        nc.scalar.activation(ksp[64:73, hsl], kps[64:73, :],
                                             AF.Identity, bias=bks2[64:73, 0:1])
                        nc.scalar.activation(ksp[0:9, hsl], kps[0:9, :],
                                             AF.Identity, bias=bks2[0:9, 0:1])
                        # channel sum (av*64) for both batches -> rows 0-1
                        avps = psA.tile([128, HF], F32, tag="psA", name="avps")
                        for q in range(2):
                            qg = 2 * h2 + q
                            MM(avps[0:2, q * Q:(q + 1) * Q], wmean2[:],
                               p1v[:, r0 + 4 * qg + 1:r0 + 4 * qg + 5, 2:130],
                               start=True, stop=True, tile_position=(0, 0))
                        nc.vector.tensor_copy(
                            mscv[0:2, r0 + 8 * h2 + 1:r0 + 8 * h2 + 9, 2:130],
                            avps[0:2, :].rearrange("p (r w) -> p r w", w=W))

                # ---- channel-max fold for pair (ch-1, ch), odd ch ----
                if ch < NCH and ch % 2 == 1:
                    seg = slice((RC * (ch - 1) + 1) * PW,
                                (RC * (ch - 1) + 1) * PW + N2)
                    sdma(vs[0:32, 0:N2], pad1[32:64, seg])
                    pdma(vs[64:96, 0:N2], pad1[96:128, seg])
                    nc.vector.tensor_max(sA[0:96, :], pad1[0:96, seg],
                                         vs[0:96, :])
                    cur, nxt = sA, sB
                    for i, k in enumerate((16, 8, 4, 2, 1)):
                        dmaf = sdma if i % 2 == 0 else pdma
                        dmaf(vs[0:k, 0:N2], cur[k:2 * k, :])
                        dmaf(vs[64:64 + k, 0:N2], cur[64 + k:64 + 2 * k, :])
                        nc.vector.tensor_max(nxt[0:64 + k, :],
                                             cur[0:64 + k, :],
                                             vs[0:64 + k, :])
                        cur, nxt = nxt, cur
                    pdma(mssc[2:3, seg], cur[0:1, :])
                    pdma(mssc[3:4, seg], cur[64:65, :])

                # ---- tap rows for chunk ch-2 (4 overlapping-AP DMAs) ----
                if 0 <= ch - 2 < NCH:
                    cht = ch - 2
                    rp = RC * cht
                    tp = r2.tile([12, NT], BF16, tag="tap")
                    taps[cht] = tp
                    for g, srow in enumerate((2, 0, 3, 1)):
                        src = strided_src(mssc, srow, rp * PW,
                                          [[PW, 3], [1, NT]])
                        pdma(tp[g * 3:(g + 1) * 3, :], src)

                # ---- ddf for chunk ch-3 ----
                if ch >= 3:
                    chd = ch - 3
                    rp = RC * chd
                    ksp = ksps.pop(chd)
                    outst = r2.tile([128, F], BF16, tag="outst")

                    for h2 in range(2):
                        rb = rp + 8 * h2

                        def kbc(t):
                            kt = psA.tile([128, HF], F32, tag="psA",
                                          name="kt")
                            for q in range(2):
                                qs = slice(q * Q, (q + 1) * Q)
                                xs = slice(h2 * HF + q * Q,
                                           h2 * HF + (q + 1) * Q)
                                if t < KK:
                                    ksl = slice(t * 64, (t + 1) * 64)
                                    MM(kt[0:64, qs], kcsel[64:73, ksl],
                                       ksp[64:73, xs], start=True, stop=True,
                                       tile_position=(64, 0))
                                    MM(kt[64:128, qs], kcsel[0:9, ksl],
                                       ksp[0:9, xs], start=True, stop=True,
                                       tile_position=(0, 64))
                                else:      # sa broadcast: 0.5*tanh
                                    MM(kt[0:64, qs], selsa[96:98, 0:64],
                                       ksp[96:98, xs], start=True, stop=True,
                                       tile_position=(96, 0))
                                    MM(kt[64:128, qs], selsa[96:98, 64:128],
                                       ksp[96:98, xs], start=True, stop=True,
                                       tile_position=(96, 64))
                            return kt

                        def padview(t):
                            if t < KK:
                                i, j = divmod(t, 3)
                            else:
                                i, j = 1, 1
                            if j == 1:
                                srcv, joff = p1v, 2
                            else:
                                srcv, joff = p2v, j + 2
                            return srcv[:, rb + i:rb + i + 8, joff:joff + W]

                        fq = [psF.tile([128, Q], F32, tag="fps", name="fq")
                              for _ in range(2)]
                        kb = {0: kbc(0), 1: kbc(1)}
                        t2 = None
                        for t in range(KK + 1):
                            if t + 2 <= KK:
                                kb[t + 2] = kbc(t + 2)
                            kt = kb.pop(t)
                            xv = padview(t)
                            if t == KK:
                                # t2 = (saB + attps) * x0_center
                                t2 = r2.tile([128, HF], BF16, tag="t2")
                                nc.vector.scalar_tensor_tensor(
                                    t2.rearrange("p (r w) -> p r w", w=W),
                                    kt.rearrange("p (r w) -> p r w", w=W),
                                    attps[:, 0:1], xv, OP.add, OP.mult)
                                break
                            idx = h2 * 9 + t
                            z = r3.tile([128, HF], BF16, tag="z")
                            zv = z.rearrange("p (r w) -> p r w", w=W)
                            if idx in GP_MUL or idx in ACT_DVE:
                                kb2 = r3.tile([128, HF], BF16, tag="kb2")
                                nc.scalar.activation(kb2[:], kt[:], AF.Copy)
                                kv = kb2.rearrange("p (r w) -> p r w", w=W)
                                if idx in GP_MUL:
                                    nc.gpsimd.tensor_tensor(zv, xv, kv,
                                                            OP.mult)
                                else:
                                    nc.vector.tensor_mul(zv, xv, kv)
                            else:
                                nc.vector.tensor_mul(
                                    zv, xv,
                                    kt.rearrange("p (r w) -> p r w", w=W))
                            for q in range(2):
                                qs = slice(q * Q, (q + 1) * Q)
                                MM(fq[q][0:64, :], wconv[0:64, :],
                                   z[0:64, qs], start=(t == 0),
                                   stop=(t == KK - 1), tile_position=(0, 0),
                                   skip_group_check=True)
                                MM(fq[q][64:128, :], wconv[64:128, :],
                                   z[64:128, qs], start=(t == 0),
                                   stop=(t == KK - 1),
                                   tile_position=(64, 64),
                                   skip_group_check=True)
                        # final: out = (fps + bconv) + t2
                        for q in range(2):
                            osl = slice(h2 * HF + q * Q, h2 * HF + (q + 1) * Q)
                            nc.vector.scalar_tensor_tensor(
                                outst[:, osl], fq[q][:], bconv[:, 0:1],
                                t2[:, q * Q:(q + 1) * Q], OP.add, OP.add)
                    sdma(out_d[:, chd * F:(chd + 1) * F], outst[:])

                # ---- sa conv + tanh for chunk ch-2 (emitted last) ----
                if 0 <= ch - 2 < NCH:
                    cht = ch - 2
                    tp = taps[cht]
                    tapv = tp.rearrange("p (r w) -> p r w", w=PW)
                    kspt = ksps[cht]
                    for h2 in range(2):
                        saps = psA.tile([128, HF], F32, tag="psA", name="saps")
                        for q in range(2):
                            qg = 2 * h2 + q
                            for j in range(3):
                                MM(saps[96:98, q * Q:(q + 1) * Q],
                                   wsa12[:, 2 * j:2 * j + 2],
                                   tapv[:, 4 * qg:4 * qg + 4, j + 1:j + 1 + W],
                                   start=(j == 0), stop=(j == 2),
                                   tile_position=(0, 96))
                        # sigmoid(x+b) = 0.5*tanh(0.5x+0.5b)+0.5; 0.5 scale is
                        # in selsa, +0.5 rides attps
                        nc.scalar.activation(
                            kspt[96:98, h2 * HF:(h2 + 1) * HF], saps[96:98, :],
                            AF.Tanh, bias=bsa[96:98, 0:1], scale=0.5)

    nc.compile()
    return nc


_CACHED = {}


def _get_program():
    if "nc" not in _CACHED:
        _CACHED["nc"] = _build_program()
    return _CACHED["nc"]


def make_in_maps(x0, x1, x2, Wkc1, Wkc2, Wks1, bks1, Wks2, bks2,
                 Wconv, bconv, Wca1, Wca2, Wsa, bsa):
    bf = ml_dtypes.bfloat16
    x0 = np.asarray(x0, np.float32)
    x1 = np.asarray(x1, np.float32)
    x2 = np.asarray(x2, np.float32)

    kc = (_leaky(x1 @ np.asarray(Wkc1, np.float32))
          @ np.asarray(Wkc2, np.float32)).reshape(B, C, KK)
    att = 1.0 / (1.0 + np.exp(-(_leaky(x1 @ np.asarray(Wca1, np.float32))
                                @ np.asarray(Wca2, np.float32))))

    wks1_np = np.tile(np.asarray(Wks1, np.float32), (2, 1)).astype(bf)
    wks2_np = np.tile(np.asarray(Wks2, np.float32), (2, 1)).astype(bf)
    wconv_np = np.tile(np.asarray(Wconv, np.float32), (2, 1)).astype(bf)

    selsa_np = np.zeros((128, 128), np.float32)
    selsa_np[96, 0:64] = 0.5     # b0 lane: ksp row 96 (tanh b0)
    selsa_np[97, 64:128] = 0.5   # b1 lane: ksp row 97 (tanh b1)
    selsa_np = selsa_np.astype(bf)

    wmean2_np = np.zeros((128, 2), np.float32)
    wmean2_np[0:64, 0] = 1.0
    wmean2_np[64:128, 1] = 1.0
    wmean2_np = wmean2_np.astype(bf)

    # sa tap weights, rows (g, i), one [12, 2] block per j shift;
    # g = (mx_b0, av_b0, mx_b1, av_b1); av rows carry 1/64 (maps hold sums)
    wsa_np = np.asarray(Wsa, np.float32)[0]          # [2 src, 3 i, 3 j]
    w12 = np.zeros((12, 6), np.float32)
    for g in range(4):
        b, srcm = divmod(g, 2)
        sc = (1.0 / 64.0) if srcm == 1 else 1.0
        for i in range(3):
            for j in range(3):
                w12[g * 3 + i, 2 * j + b] = wsa_np[srcm, i, j] * sc
    w12 = w12.astype(bf)

    bks1_np = np.tile(np.asarray(bks1, np.float32), 2).reshape(128, 1)
    bks2_np = np.zeros((128, 1), np.float32)
    bks2_np[0:9, 0] = np.asarray(bks2, np.float32)
    bks2_np[64:73, 0] = np.asarray(bks2, np.float32)
    bsa_np = np.full((128, 1), 0.5 * float(np.asarray(bsa, np.float32)[0]),
                     np.float32)
    bconv_np = np.ascontiguousarray(
        np.tile(np.asarray(bconv, np.float32), 2).reshape(128, 1))

    shared = {
        "wks1": wks1_np, "wks2": wks2_np, "wconv": wconv_np,
        "selsa": selsa_np, "wmean2": wmean2_np, "wsa12": w12,
        "bks1": bks1_np, "bks2": bks2_np, "bsa": bsa_np,
        "bconv": bconv_np,
    }

    in_maps = []
    for cid in range(NCORES):
        bsl = slice(BPC * cid, BPC * (cid + 1))
        m = dict(shared)
        m["x0b"] = np.ascontiguousarray(x0[bsl].reshape(128, HW).astype(bf))
        m["x2b"] = np.ascontiguousarray(x2[bsl].reshape(128, HW).astype(bf))
        # kc selector + folded ksp-bias row: b0 rows 64-73, b1 rows 0-9
        kcs = np.zeros((128, KK * 64), np.float32)
        for ij in range(KK):
            kcs[64 + ij, ij * 64:(ij + 1) * 64] = kc[BPC * cid, :, ij]
            kcs[ij, ij * 64:(ij + 1) * 64] = kc[BPC * cid + 1, :, ij]
        m["kcsel"] = kcs.astype(bf)
        ap = np.empty((128, 1), np.float32)
        ap[0:64, 0] = att[BPC * cid] + 1.5
        ap[64:128, 0] = att[BPC * cid + 1] + 1.5
        m["attps"] = ap
        in_maps.append(m)
    return in_maps


def kernel(**inputs):
    in_maps = make_in_maps(**inputs)
    nc = _get_program()
    res = run_bass_kernel_spmd(nc, in_maps, list(range(NCORES)))
    out = np.empty((B, C, H, W), np.float32)
    for cid in range(NCORES):
        out[BPC * cid:BPC * (cid + 1)] = \
            res.results[cid]["outb"].astype(np.float32).reshape(BPC, C, H, W)
    return out


if __name__ == "__main__":
    _get_program()
    print("program built and compiled OK")
